# revision 1
# baseline (speedup 1.0000x reference)
"""Bass/Tile TRN2 kernel for nn_BilateralCostVolume.

For each of 81 displacements d=(du,dv) and batch b:
    out[b,r,h,w] = <bilinear(f2n, p + (BM+d)), bilinear(f1n, p - (BM+d))> * mask
where f1n/f2n are channel-l2-normalized features, sampling matches
F.grid_sample(align_corners=False, border padding), and the zeros-padding
validity mask is binarized at 0.999.

Sharding: 162 (b, r) planes over 8 cores.  Slot positions have a STATIC
batch: slots 0..10 hold b=0 planes, slots 11..21 hold b=1 planes (padded with
duplicates), so each slot's gathers read a compile-time table tensor.

Per core:
  1. normalize both features for both batches, build 4 "quad tables" in DRAM:
     row (y,x) = [F[y,x], F[y,x+1], F[y+1,x], F[y+1,x+1]] (edge-clamped),
     192 f32 = 768 B per row;
  2. per plane: compute sample coords / bilinear weights / masks as
     [w=128, h=80] fields on DVE/ACT; build the wrapped int16 index layout
     for dma_gather via a DRAM round-trip; gather 2x2 patches per pixel with
     nc.gpsimd.dma_gather (768 B per index); weighted-sum the 4 corners
     (weights enter as step-0 broadcast APs), channel-dot, mask, transpose,
     store.
"""

import numpy as np

import concourse.bass as bass
import concourse.bacc as bacc
import concourse.mybir as mybir
import concourse.tile as tile
from concourse import bass_utils
from concourse.masks import make_identity

MD = 4
R = (2 * MD + 1) ** 2  # 81
B, C, H, W = 2, 48, 80, 128
HWPIX = H * W
SW = float(W) / float(W - 1)
SH = float(H) / float(H - 1)
NCORES = 8
NSB = 11          # slots per batch half (8*11 = 88 >= 81)
NSLOT = 2 * NSB   # 22
CH = 40           # h-chunk size (2 chunks per plane)
QW = 4 * C        # quad patch payload (192 elements)
TQW = QW          # table row width (f32, 768 B rows)

F32 = mybir.dt.float32
I32 = mybir.dt.int32
I16 = mybir.dt.int16
BF16 = mybir.dt.bfloat16
AF = mybir.ActivationFunctionType
OP = mybir.AluOpType

NSC = 4  # per-slot scalar columns: cfx, cbx, cfy, cby


def _plan():
    """Slots 0..NSB-1 are b=0 planes, NSB..2NSB-1 are b=1 planes."""
    counts = {0: [11, 10, 10, 10, 10, 10, 10, 10],
              1: [11, 10, 10, 10, 10, 10, 10, 10]}
    slots_per_core = []   # list of NSLOT (b, r)
    valid_per_core = []   # list of NSLOT bool
    for k in range(NCORES):
        slots, valid = [], []
        for b in (0, 1):
            start = sum(counts[b][:k])
            rs = list(range(start, start + counts[b][k]))
            v = [True] * len(rs)
            while len(rs) < NSB:
                rs.append(rs[-1])
                v.append(False)
            slots += [(b, r) for r in rs]
            valid += v
        slots_per_core.append(slots)
        valid_per_core.append(valid)
    return slots_per_core, valid_per_core


def _lin():
    return np.linspace(-MD, MD, 2 * MD + 1).astype(np.float64)


def build_program(dbg=False):
    nc = bacc.Bacc(
        "TRN2",
        target_bir_lowering=False,
        debug=False,
        enable_asserts=False,
        num_devices=NCORES,
        num_swdge_queues=2,
    )

    f1_d = nc.dram_tensor("f1", [B, C, H, W], F32, kind="ExternalInput")
    f2_d = nc.dram_tensor("f2", [B, C, H, W], F32, kind="ExternalInput")
    bmp_d = nc.dram_tensor("bmp", [NSLOT, 128, 2 * H], F32,
                           kind="ExternalInput")
    sc_d = nc.dram_tensor("sc", [128, NSLOT * NSC], F32, kind="ExternalInput")
    wio_d = nc.dram_tensor("wio", [128, 1], F32, kind="ExternalInput")
    hf_d = nc.dram_tensor("hf", [128, H], F32, kind="ExternalInput")
    out_d = nc.dram_tensor("out", [NSLOT, H, W], F32, kind="ExternalOutput")

    with tile.TileContext(nc) as tc:
        with (
            tc.tile_pool(name="const", bufs=1) as constp,
            tc.tile_pool(name="dram", bufs=1, space="DRAM") as dramp,
        ):
            ident = constp.tile([128, 128], F32)
            make_identity(nc, ident[:])
            eps = constp.tile([128, 1], F32)
            nc.gpsimd.memset(eps[:], 1e-6)
            wio = constp.tile([128, 1], F32)
            nc.sync.dma_start(out=wio[:], in_=wio_d.ap())
            hf = constp.tile([128, H], F32)
            nc.sync.dma_start(out=hf[:], in_=hf_d.ap())
            sc = constp.tile([128, NSLOT * NSC], F32)
            nc.sync.dma_start(out=sc[:], in_=sc_d.ap())

            # tabs[f][b]; f=0 -> feature1 (bw warp), f=1 -> feature2 (fw)
            t10 = dramp.tile([HWPIX, TQW], F32)
            t11 = dramp.tile([HWPIX, TQW], F32)
            t20 = dramp.tile([HWPIX, TQW], F32)
            t21 = dramp.tile([HWPIX, TQW], F32)
            tabs = [[t10, t11], [t20, t21]]

            # ---------------- Phase 1: normalize + quad tables -------------
            with (
                tc.tile_pool(name="fc", bufs=1) as fcp,
                tc.tile_pool(name="qt", bufs=1) as qtp,
                tc.tile_pool(name="ps", bufs=2, space="PSUM") as psp,
            ):
                WH = W // 2
                for tabi in range(4):
                    f = tabi // 2
                    b = tabi % 2
                    qt = qtp.tile([H, W, 4, C], F32, tag="qt")
                    for wh in range(2):  # w halves to bound SBUF
                        src = (f1_d if f == 0 else f2_d).ap()[b]
                        src = src[:, :, wh * WH:(wh + 1) * WH]  # [C, H, WH]
                        fc = fcp.tile([C, H, WH], F32, tag="fc")
                        nc.sync.dma_start(out=fc[:], in_=src)
                        for j in range(WH // 8):
                            pt = psp.tile([H, 8 * C], F32, tag="pt")
                            for jj in range(8):
                                w = 8 * j + jj
                                nc.tensor.transpose(
                                    out=pt[:, jj * C:(jj + 1) * C],
                                    in_=fc[:, :, w],
                                    identity=ident[:C, :C],
                                )
                            wg = wh * WH + 8 * j
                            if j % 2 == 0:
                                nc.vector.tensor_copy(
                                    qt[:, wg:wg + 8, 0, :], pt[:])
                            else:
                                nc.scalar.copy(
                                    qt[:, wg:wg + 8, 0, :], pt[:])

                    # normalize over c: squares into qt slot 1 (scratch)
                    nc.scalar.activation(
                        qt[:, :, 1, :], qt[:, :, 0, :], AF.Square)
                    ssq = fcp.tile([H, W], F32, tag="ssq")
                    nc.vector.tensor_reduce(
                        ssq[:], qt[:, :, 1, :], axis=mybir.AxisListType.X,
                        op=OP.add)
                    rn = fcp.tile([H, W], F32, tag="rn")
                    nc.scalar.activation(
                        rn[:], ssq[:], AF.Sqrt, bias=eps[:H, :])
                    nc.vector.reciprocal(rn[:], rn[:])
                    nc.scalar.copy(
                        qt[:, :, 1, :],
                        rn[:].unsqueeze(-1).broadcast_to([H, W, C]))
                    nc.vector.tensor_mul(
                        qt[:, :, 0, :], qt[:, :, 0, :], qt[:, :, 1, :])

                    # x-shift into slot 1 (from normalized slot 0)
                    nc.scalar.copy(qt[:, 0:W - 1, 1, :], qt[:, 1:W, 0, :])
                    nc.scalar.copy(qt[:, W - 1, 1, :], qt[:, W - 1, 0, :])
                    # y-shift via SBUF->SBUF DMA (partition shift)
                    nc.sync.dma_start(
                        out=qt[0:H - 1, :, 2:4, :], in_=qt[1:H, :, 0:2, :])
                    nc.sync.dma_start(
                        out=qt[H - 1:H, :, 2:4, :], in_=qt[H - 1:H, :, 0:2, :])

                    # write table rows [HWPIX, QW]
                    dst = tabs[f][b][:]
                    dst = dst.rearrange("(h w) q -> h (w q)", h=H)
                    nc.sync.dma_start(
                        out=dst, in_=qt[:].rearrange("h w a c -> h (w a c)"))

            # ---------------- Phase 2: per-plane slots ----------------------
            with (
                tc.tile_pool(name="fld", bufs=2) as fld,
                tc.tile_pool(name="pre", bufs=3) as prep,
                tc.tile_pool(name="iscr", bufs=2, space="DRAM") as iscrp,
                tc.tile_pool(name="ops", bufs=2, space="PSUM") as psp2,
            ):
                for s in range(NSLOT):
                    sb = 0 if s < NSB else 1
                    tabF = tabs[1][sb][:]   # f2 quad table
                    tabB = tabs[0][sb][:]   # f1 quad table
                    col = lambda j: sc[:, s * NSC + j:s * NSC + j + 1]
                    cfx, cbx, cfy, cby = (col(j) for j in range(NSC))

                    bmp = fld.tile([128, 2 * H], F32, tag="bmp")
                    nc.sync.dma_start(out=bmp[:], in_=bmp_d.ap()[s])
                    bmx = bmp[:, 0:H]
                    bmy = bmp[:, H:2 * H]

                    # sample coordinates, both warps batched in one
                    # [128, 2H] field: cols 0:H warp F, H:2H warp B
                    H2 = 2 * H
                    t2 = lambda tg: fld.tile([128, H2], F32, tag=tg, name=tg)
                    ix2 = t2("ix2")
                    nc.vector.tensor_scalar(
                        out=ix2[:, 0:H], in0=bmx, scalar1=wio[:], scalar2=SW,
                        op0=OP.add, op1=OP.mult)
                    nc.vector.tensor_scalar(
                        out=ix2[:, 0:H], in0=ix2[:, 0:H], scalar1=cfx,
                        scalar2=None, op0=OP.add)
                    nc.vector.tensor_scalar(
                        out=ix2[:, H:H2], in0=bmx, scalar1=wio[:],
                        scalar2=-SW, op0=OP.subtract, op1=OP.mult)
                    nc.vector.tensor_scalar(
                        out=ix2[:, H:H2], in0=ix2[:, H:H2], scalar1=cbx,
                        scalar2=None, op0=OP.add)
                    iy2 = t2("iy2")
                    tmy = fld.tile([128, H], F32, tag="tmy")
                    nc.vector.tensor_add(tmy[:], bmy, hf[:])
                    nc.vector.tensor_scalar(
                        out=iy2[:, 0:H], in0=tmy[:], scalar1=SH, scalar2=cfy,
                        op0=OP.mult, op1=OP.add)
                    nc.vector.tensor_sub(tmy[:], hf[:], bmy)
                    nc.vector.tensor_scalar(
                        out=iy2[:, H:H2], in0=tmy[:], scalar1=SH, scalar2=cby,
                        op0=OP.mult, op1=OP.add)

                    ixc = t2("ixc")
                    nc.vector.tensor_scalar(
                        out=ixc[:], in0=ix2[:], scalar1=0.0,
                        scalar2=float(W - 1), op0=OP.max, op1=OP.min)
                    iyc = t2("iyc")
                    nc.vector.tensor_scalar(
                        out=iyc[:], in0=iy2[:], scalar1=0.0,
                        scalar2=float(H - 1), op0=OP.max, op1=OP.min)

                    def floorfrac(srcf, f0tag, frtag):
                        # floor for srcf >= 0, robust to convert rounding
                        xi = fld.tile([128, H2], I32, tag=f0tag + "i",
                                      name=f0tag + "i")
                        nc.vector.tensor_copy(xi[:], srcf[:])
                        xf = t2(f0tag + "f")
                        nc.vector.tensor_copy(xf[:], xi[:])
                        er = t2(f0tag + "e")
                        nc.vector.tensor_tensor(
                            out=er[:], in0=xf[:], in1=srcf[:], op=OP.is_gt)
                        f0 = t2(f0tag)
                        nc.vector.tensor_sub(f0[:], xf[:], er[:])
                        fr = t2(frtag)
                        nc.vector.tensor_sub(fr[:], srcf[:], f0[:])
                        return f0, fr

                    x0, wx = floorfrac(ixc, "x0", "wx")
                    y0, wy = floorfrac(iyc, "y0", "wy")
                    xcf = t2("xcf")
                    nc.vector.tensor_scalar(
                        out=xcf[:], in0=x0[:], scalar1=float(W - 2),
                        scalar2=None, op0=OP.min)
                    bx = t2("bx")
                    nc.vector.tensor_sub(bx[:], x0[:], xcf[:])
                    nc.vector.tensor_add(bx[:], bx[:], wx[:])
                    # gather row index = y0*W + xc  (fits int16)
                    idxf = t2("idxf")
                    nc.vector.scalar_tensor_tensor(
                        out=idxf[:], in0=y0[:], scalar=float(W), in1=xcf[:],
                        op0=OP.mult, op1=OP.add)
                    # mask (trapezoid per axis)
                    ma = t2("ma")
                    nc.vector.tensor_scalar(
                        out=ma[:], in0=ix2[:], scalar1=-1.0,
                        scalar2=float(W), op0=OP.mult, op1=OP.add)
                    mb = t2("mb")
                    nc.vector.tensor_scalar(
                        out=mb[:], in0=ix2[:], scalar1=1.0, scalar2=None,
                        op0=OP.add)
                    nc.vector.tensor_tensor(
                        out=ma[:], in0=ma[:], in1=mb[:], op=OP.min)
                    nc.vector.tensor_scalar(
                        out=ma[:], in0=ma[:], scalar1=0.0, scalar2=1.0,
                        op0=OP.max, op1=OP.min)
                    mc = t2("mc")
                    nc.vector.tensor_scalar(
                        out=mc[:], in0=iy2[:], scalar1=-1.0,
                        scalar2=float(H), op0=OP.mult, op1=OP.add)
                    md = t2("md")
                    nc.vector.tensor_scalar(
                        out=md[:], in0=iy2[:], scalar1=1.0, scalar2=None,
                        op0=OP.add)
                    nc.vector.tensor_tensor(
                        out=mc[:], in0=mc[:], in1=md[:], op=OP.min)
                    nc.vector.tensor_scalar(
                        out=mc[:], in0=mc[:], scalar1=0.0, scalar2=1.0,
                        op0=OP.max, op1=OP.min)
                    msk2 = t2("msk2")
                    nc.vector.tensor_mul(msk2[:], ma[:], mc[:])
                    # bilinear corner weights
                    uy = t2("uy")
                    nc.vector.tensor_scalar(
                        out=uy[:], in0=wy[:], scalar1=-1.0, scalar2=1.0,
                        op0=OP.mult, op1=OP.add)
                    vx = t2("vx")
                    nc.vector.tensor_scalar(
                        out=vx[:], in0=bx[:], scalar1=-1.0, scalar2=1.0,
                        op0=OP.mult, op1=OP.add)
                    wa = t2("wa")
                    nc.vector.tensor_mul(wa[:], uy[:], vx[:])
                    wb = t2("wb")
                    nc.vector.tensor_mul(wb[:], uy[:], bx[:])
                    wc = t2("wc")
                    nc.vector.tensor_mul(wc[:], wy[:], vx[:])
                    wd = t2("wd")
                    nc.vector.tensor_mul(wd[:], wy[:], bx[:])

                    # per-warp: int16 idx + wrapped layout via DRAM trip
                    wrs = []
                    for u, wtag in enumerate(("F", "Bw")):
                        idx16 = fld.tile([128, H], I16, tag=wtag + "idx16",
                                         name=wtag + "idx16")
                        nc.vector.tensor_copy(
                            idx16[:], idxf[:, u * H:(u + 1) * H])
                        iscr = iscrp.tile([16, 8 * H], I16,
                                          tag=wtag + "iscr",
                                          name=wtag + "iscr")
                        nc.sync.dma_start(
                            out=iscr[:].rearrange("q (h m) -> m q h", m=8),
                            in_=idx16[:])
                        wr = fld.tile([128, 8 * H], I16, tag=wtag + "wr",
                                      name=wtag + "wr")
                        nc.sync.dma_start(
                            out=wr[:],
                            in_=iscr[:].unsqueeze(0).broadcast_to(
                                [8, 16, 8 * H]))
                        wrs.append(wr)
                    wrF, wrB = wrs
                    wgt2 = (wa, wb, wc, wd)
                    wF = tuple(w[:, 0:H] for w in wgt2)
                    wB = tuple(w[:, H:H2] for w in wgt2)

                    mall = fld.tile([128, H], F32, tag="mall")
                    nc.vector.tensor_mul(
                        mall[:], msk2[:, 0:H], msk2[:, H:H2])
                    nc.vector.tensor_scalar(
                        out=mall[:], in0=mall[:], scalar1=0.999, scalar2=None,
                        op0=OP.is_ge)

                    acc = fld.tile([128, H], F32, tag="acc")

                    for c0 in range(0, H, CH):
                        pres = []
                        for wi, (wrt, tabt, wgt) in enumerate(
                                ((wrF, tabF, wF), (wrB, tabB, wB))):
                            eng = nc.vector if wi == 0 else nc.gpsimd
                            pre = prep.tile(
                                [128, CH, TQW], F32, tag="pre",
                                name=f"pre{wi}")
                            off = c0 * 8
                            nc.gpsimd.dma_gather(
                                out_ap=pre[:],
                                in_ap=tabt,
                                idxs_ap=wrt[:, off:off + CH * 8],
                                num_idxs=CH * 128,
                                num_idxs_reg=CH * 128,
                                elem_size=TQW,
                                single_packet=False,
                                queue_num=wi,
                            )
                            # weighted sum of 4 corners (in place); weights
                            # enter as step-0 broadcast APs over channels.
                            for q in range(4):
                                wq = wgt[q][:, c0:c0 + CH]
                                wq = wq.unsqueeze(-1).broadcast_to(
                                    [128, CH, C])
                                eng.tensor_mul(
                                    pre[:, :, q * C:(q + 1) * C],
                                    pre[:, :, q * C:(q + 1) * C], wq)
                            eng.tensor_add(
                                pre[:, :, 0:2 * C], pre[:, :, 0:2 * C],
                                pre[:, :, 2 * C:4 * C])
                            eng.tensor_add(
                                pre[:, :, 0:C], pre[:, :, 0:C],
                                pre[:, :, C:2 * C])
                            pres.append(pre)
                        # channel dot
                        nc.vector.tensor_mul(
                            pres[0][:, :, 0:C], pres[0][:, :, 0:C],
                            pres[1][:, :, 0:C])
                        nc.vector.tensor_reduce(
                            acc[:, c0:c0 + CH], pres[0][:, :, 0:C],
                            axis=mybir.AxisListType.X, op=OP.add)

                    nc.vector.tensor_mul(acc[:], acc[:], mall[:])

                    # transpose [128, 80] -> [80, 128] and store
                    pt2 = psp2.tile([H, 128], F32, tag="pt2")
                    nc.tensor.transpose(
                        out=pt2[:], in_=acc[:], identity=ident[:])
                    ot = fld.tile([H, W], F32, tag="ot")
                    nc.scalar.copy(ot[:], pt2[:])
                    nc.sync.dma_start(out=out_d.ap()[s], in_=ot[:])

    nc.compile()
    return nc


def make_in_maps(feature1, feature2, BM):
    """Build the 8 per-core input maps from full inputs."""
    slots_per_core, valid_per_core = _plan()
    lin = _lin()
    f1 = np.ascontiguousarray(np.asarray(feature1, dtype=np.float32))
    f2 = np.ascontiguousarray(np.asarray(feature2, dtype=np.float32))
    bm = np.asarray(BM, dtype=np.float32)

    wio = np.arange(W, dtype=np.float32).reshape(128, 1)
    hfv = np.broadcast_to(
        np.arange(H, dtype=np.float32)[None, :], (128, H)).copy()

    in_maps = []
    for k in range(NCORES):
        slots = slots_per_core[k]
        bmp = np.zeros((NSLOT, 128, 2 * H), np.float32)
        sc = np.zeros((128, NSLOT * NSC), np.float32)
        for s, (b, r) in enumerate(slots):
            du = lin[r % (2 * MD + 1)]
            dv = lin[r // (2 * MD + 1)]
            bmp[s, :, 0:H] = bm[b, 0].T  # [w, h]
            bmp[s, :, H:2 * H] = bm[b, 1].T
            sc[:, s * NSC + 0] = np.float32(du * SW - 0.5)   # cfx
            sc[:, s * NSC + 1] = np.float32(-du * SW - 0.5)  # cbx
            sc[:, s * NSC + 2] = np.float32(dv * SH - 0.5)   # cfy
            sc[:, s * NSC + 3] = np.float32(-dv * SH - 0.5)  # cby
        in_maps.append({
            "f1": f1, "f2": f2,
            "bmp": bmp, "sc": sc,
            "wio": wio, "hf": hfv,
        })
    return in_maps, slots_per_core, valid_per_core


_NC_CACHE = {}


def get_program():
    if "nc" not in _NC_CACHE:
        _NC_CACHE["nc"] = build_program()
    return _NC_CACHE["nc"]


def assemble_output(results, slots_per_core, valid_per_core):
    out = np.zeros((B, R, H, W), np.float32)
    for k in range(NCORES):
        core_out = results[k]["out"]  # [NSLOT, H, W]
        for s in range(NSLOT):
            if valid_per_core[k][s]:
                b, r = slots_per_core[k][s]
                out[b, r] = core_out[s]
    return out


def kernel(feature1, feature2, BM):
    nc = get_program()
    in_maps, slots_per_core, valid_per_core = make_in_maps(
        feature1, feature2, BM)
    res = bass_utils.run_bass_kernel_spmd(
        nc, in_maps, core_ids=list(range(NCORES)))
    return assemble_output(res.results, slots_per_core, valid_per_core)



# revision 21
# speedup vs baseline: 2.9348x; 2.9348x over previous
"""Bass/Tile TRN2 kernel for nn_BilateralCostVolume — windowed-gather design.

out[b,r,h,w] = <bilinear(f2n, p + d_r), bilinear(f1n, p - d_r)> * mask
with d_r = BM + (du,dv), r = dv*9+du, du/dv in linspace(-4,4,9), t=0.5.

Key idea: for each pixel, the 81 displacement samples of one warp all lie in
an 11x11 window around a per-pixel center (BM enters the center; du/dv are
integer-ish shifts).  Gather that window ONCE per (pixel, warp) — 11 rows of
12px x 64c bf16 (1536 B descriptors) from a padded, edge-replicated,
channel-last table — then do separable interpolation shared across
displacements:

  x-stage: XI[du, py, c] = sum_t wx4[du,t] * win[py, du+t, c]   (4 taps)
  y-stage: FW[dv, du, c] = sum_k wy3[dv,k] * XI[du, dv+k, c]    (3 taps)
  dot:     out[r] = sum_c FWF * FWB  (warp B du-axis pre-flipped), * mask

Weights / gather indices / masks are host-precomputed from BM (f32 math
mirroring the reference).  Tables are built on device: l2-normalize over c,
transpose to [y, x, c], pad, cast bf16.

Sharding: 160 (b, h) pixel rows over 8 cores; 128 pixels of a row on SBUF
partitions; each core sees only its batch's features (host slices).
"""

import numpy as np
from ml_dtypes import bfloat16

import concourse.bass as bass
import concourse.bacc as bacc
import concourse.mybir as mybir
import concourse.tile as tile
from concourse import bass_utils
from concourse.masks import make_identity

MD = 4
R = 81
B, C, H, W = 2, 48, 80, 128
SW = np.float32(W) / np.float32(W - 1)
SH = np.float32(H) / np.float32(H - 1)
CP = 64                  # padded channels in tables
XPAD = 12
YPAD = 12
XT = W + 2 * XPAD + 2    # 154 (even)
YT = H + 2 * YPAD        # 104
NPAIR = XT // 2          # 77
NROWS = YT * NPAIR       # 8008
NROWS_AL = 8016          # allocated rows (tail pad for 768-elem overrun)
NCORES = 8
GPC = 20                 # (b, h) groups per core
NW = 11                  # window rows per pixel
EL = 768                 # gather elem_size (12 px * 64 c, bf16 -> 1536 B)
ES = 128                 # gather elem_step (2 px * 64 c = 256 B)
NIDX = NW * 128          # 1408 idxs per gather

F32 = mybir.dt.float32
I16 = mybir.dt.int16
BF16 = mybir.dt.bfloat16
AF = mybir.ActivationFunctionType
OP = mybir.AluOpType

LIN = np.linspace(-MD, MD, 2 * MD + 1).astype(np.float32)


# ------------------------------------------------------------------ program
def _overlap_ap(t_ap, offset_elems=0):
    """View a [NROWS_AL, ES] dram tile as overlapped gather rows
    [[ES, NROWS_AL], [1, EL]]."""
    a = t_ap.copy()
    v = a.ap
    v.clear()
    v.extend([(ES, NROWS_AL - 5), (1, EL)])
    a.offset = a.offset + offset_elems
    return a


def build_program(dbg=False):
    nc = bacc.Bacc(
        "TRN2",
        target_bir_lowering=False,
        debug=False,
        enable_asserts=False,
        num_devices=NCORES,
        num_swdge_queues=2,
    )

    f2b_d = nc.dram_tensor("f2b", [C, H, W], F32, kind="ExternalInput")
    f1b_d = nc.dram_tensor("f1b", [C, H, W], F32, kind="ExternalInput")
    wxy_d = nc.dram_tensor("wxy", [GPC, 128, 126], F32, kind="ExternalInput")
    msk_d = nc.dram_tensor("msk", [GPC, 128, R], F32, kind="ExternalInput")
    gidx_d = nc.dram_tensor("gidx", [GPC, 2, 128, 88], I16,
                            kind="ExternalInput")
    out_d = nc.dram_tensor("out", [GPC, 128, R], F32, kind="ExternalOutput")

    with tile.TileContext(nc) as tc:
        with (
            tc.tile_pool(name="const", bufs=1) as constp,
            tc.tile_pool(name="dram", bufs=1, space="DRAM") as dramp,
        ):
            ident = constp.tile([128, 128], F32)
            make_identity(nc, ident[:])

            tabF = dramp.tile([NROWS_AL, ES], BF16)   # f2n table (warp F)
            tabB = dramp.tile([NROWS_AL, ES], BF16)   # f1n table (warp B)

            # ---------------- Phase 1: normalize + padded tables ----------
            for src_d, tab in ((f2b_d, tabF), (f1b_d, tabB)):
                with (
                    tc.tile_pool(name="p1", bufs=1) as p1,
                    tc.tile_pool(name="ps1", bufs=2, space="PSUM") as ps1,
                ):
                    fc = p1.tile([C, H * W], F32, tag="fc")
                    nc.sync.dma_start(out=fc[:], in_=src_d.ap().rearrange(
                        "c h w -> c (h w)"))
                    T = p1.tile([128, H, C], F32, tag="T")
                    for hb in range(8):
                        pt = ps1.tile([128, 10 * C], F32, tag="pt")
                        for j in range(10):
                            h = hb * 10 + j
                            nc.tensor.transpose(
                                out=pt[:, j * C:(j + 1) * C],
                                in_=fc[:, h * W:(h + 1) * W],
                                identity=ident[:C, :C])
                        nc.scalar.copy(
                            T[:, hb * 10:(hb + 1) * 10, :], pt[:])
                    sq = p1.tile([128, H, C], F32, tag="sq")
                    nc.scalar.square(sq[:], T[:])
                    ssq = p1.tile([128, H], F32, tag="ssq")
                    nc.vector.tensor_reduce(
                        ssq[:], sq[:], axis=mybir.AxisListType.X, op=OP.add)
                    rn = p1.tile([128, H], F32, tag="rn")
                    nc.scalar.activation(rn[:], ssq[:], AF.Sqrt,
                                         bias=np.float32(1e-6))
                    nc.vector.reciprocal(rn[:], rn[:])
                    re = p1.tile([128, H, C], BF16, tag="re")
                    nc.vector.tensor_copy(
                        re[:], rn[:].unsqueeze(-1).broadcast_to([128, H, C]))
                    tb = p1.tile([128, H, C], BF16, tag="tb")
                    nc.vector.tensor_copy(tb[:], T[:])
                    tn = p1.tile([128, H, C], BF16, tag="tn")
                    nc.vector.tensor_mul(tn[:], tb[:], re[:])

                    # interior write: px (XPAD + w) of row (YPAD + h)
                    dst = tab[:].copy()
                    v = dst.ap
                    v.clear()
                    # dims: (w 128 part-ish? no — DMA from SBUF [128,...]
                    # source partitions = w); dst elem offset:
                    # ((YPAD+h)*XT + XPAD + w)*CP + c
                    v.extend([(CP, 128), (XT * CP, H), (1, C)])
                    dst.offset = dst.offset + (YPAD * XT + XPAD) * CP
                    nc.sync.dma_start(out=dst, in_=tn[:])

                    # x pads: left cols [0, XPAD) <- col x=0 ; right
                    # [XPAD+W, XT) <- col x=W-1  (dram->dram, bcast px)
                    for px0, npx, srcx in ((0, XPAD, 0),
                                           (XPAD + W, XT - XPAD - W, W - 1)):
                        sap = tab[:].copy()
                        v = sap.ap
                        v.clear()
                        v.extend([(XT * CP, YT - 24), (0, npx), (1, CP)])
                        sap.offset = (sap.offset
                                      + (YPAD * XT + XPAD + srcx) * CP)
                        dap = tab[:].copy()
                        v = dap.ap
                        v.clear()
                        v.extend([(XT * CP, YT - 24), (CP, npx), (1, CP)])
                        dap.offset = dap.offset + (YPAD * XT + px0) * CP
                        nc.sync.dma_start(out=dap, in_=sap)

                    # y pads: rows [0, YPAD) <- row y=0 ; [YPAD+H, YT) <- last
                    for y0, ny, srcy in ((0, YPAD, YPAD),
                                         (YPAD + H, YT - YPAD - H,
                                          YPAD + H - 1)):
                        sap = tab[:].copy()
                        v = sap.ap
                        v.clear()
                        v.extend([(0, ny), (1, XT * CP)])
                        sap.offset = sap.offset + srcy * XT * CP
                        dap = tab[:].copy()
                        v = dap.ap
                        v.clear()
                        v.extend([(XT * CP, ny), (1, XT * CP)])
                        dap.offset = dap.offset + y0 * XT * CP
                        nc.sync.dma_start(out=dap, in_=sap)

            # ---------------- Phase 2: per-group windows ------------------
            # Software-pipelined emission: loads/gathers of group g+2 and
            # mults of group g+1 are emitted before the adds/dot of group g
            # so in-order engine queues never stall on cross-engine deps.
            with (
                tc.tile_pool(name="win", bufs=2) as winp,
                tc.tile_pool(name="ld", bufs=3) as ldp,
                tc.tile_pool(name="cmp", bufs=2) as cmp_,
                tc.tile_pool(name="yst", bufs=1) as yst,
            ):
                XM_ENG = {(0, 0): "v", (0, 1): "a", (0, 2): "a",
                          (0, 3): "a", (1, 0): "a", (1, 1): "p",
                          (1, 2): "p", (1, 3): "v"}
                YM_ENG = {(0, 0): "v", (0, 1): "v", (0, 2): "a",
                          (1, 0): "a", (1, 1): "p", (1, 2): "p"}

                def mul_op(eng, out, in0, sc):
                    if eng == "a":
                        nc.scalar.mul(out, in0, sc)
                    elif eng == "p":
                        nc.gpsimd.tensor_scalar(
                            out=out, in0=in0, scalar1=sc, scalar2=None,
                            op0=OP.mult)
                    else:
                        nc.vector.tensor_scalar(
                            out=out, in0=in0, scalar1=sc, scalar2=None,
                            op0=OP.mult)

                tiles = {}

                def S0(g):
                    d = {}
                    d["wv"] = ldp.tile([128, 126], F32, tag="wv", name="wv")
                    nc.sync.dma_start(out=d["wv"][:], in_=wxy_d.ap()[g])
                    d["mk"] = ldp.tile([128, R], F32, tag="mk", name="mk")
                    nc.sync.dma_start(out=d["mk"][:], in_=msk_d.ap()[g])
                    d["wins"] = []
                    for wi, tab in enumerate((tabF, tabB)):
                        gx = ldp.tile([128, 88], I16, tag=f"gx{wi}",
                                      name=f"gx{wi}")
                        nc.sync.dma_start(out=gx[:], in_=gidx_d.ap()[g, wi])
                        win = winp.tile([128, NW, EL], BF16, tag=f"win{wi}",
                                        name=f"win{wi}")
                        nc.gpsimd.dma_gather(
                            out_ap=win[:],
                            in_ap=_overlap_ap(tab[:]),
                            idxs_ap=gx[:],
                            num_idxs=NIDX,
                            num_idxs_reg=NIDX,
                            elem_size=EL,
                            elem_step=ES,
                            single_packet=False,
                            queue_num=wi,
                        )
                        d["wins"].append(win)
                    d["TMPS"] = {}
                    d["XIs"] = {}
                    d["FWs"] = {}
                    tiles[g] = d

                def SM(g, wi):
                    """Tap-product multiplies for warp wi of group g."""
                    d = tiles[g]
                    wv = d["wv"]
                    win = d["wins"][wi]
                    TMPS = [cmp_.tile([128, 9, NW, C], BF16, tag=f"TMP{j}",
                                      name=f"TMP{j}") for j in range(3)]
                    d["TMPS"][wi] = TMPS
                    XI = cmp_.tile([128, 9, NW, C], BF16, tag=f"XI{wi}",
                                   name=f"XI{wi}")
                    d["XIs"][wi] = XI
                    wb = 63 * wi
                    for t in range(4):
                        dstt = XI if t == 0 else TMPS[t - 1]
                        eng = XM_ENG[(wi, t)]
                        for dui in range(9):
                            do = dui if wi == 0 else 8 - dui
                            mul_op(
                                eng, dstt[:, do, :, :],
                                win[:, :, (dui + t) * CP:(dui + t) * CP + C],
                                wv[:, wb + dui * 4 + t:wb + dui * 4 + t + 1])

                def SD(g, wi):
                    """x-adds for warp wi of group g (DVE)."""
                    d = tiles[g]
                    XI = d["XIs"][wi]
                    TMPS = d["TMPS"][wi]
                    nc.vector.tensor_add(TMPS[0][:], TMPS[0][:], TMPS[1][:])
                    nc.vector.tensor_add(XI[:], XI[:], TMPS[2][:])
                    nc.vector.tensor_add(XI[:], XI[:], TMPS[0][:])

                def SBW(g, wi):
                    """y-stage for warp wi of group g."""
                    d = tiles[g]
                    wv = d["wv"]
                    XI = d["XIs"][wi]
                    YTS = [yst.tile([128, 9, 9, C], BF16, tag=f"YT{j}",
                                    name=f"YT{j}") for j in range(2)]
                    FW = yst.tile([128, 9, 9, C], BF16, tag=f"FW{wi}",
                                  name=f"FW{wi}")
                    d["FWs"][wi] = FW
                    wb = 63 * wi + 36
                    for k in range(3):
                        dstt = FW if k == 0 else YTS[k - 1]
                        eng = YM_ENG[(wi, k)]
                        for dvi in range(9):
                            mul_op(
                                eng, dstt[:, dvi, :, :],
                                XI[:, :, dvi + k, :],
                                wv[:, wb + dvi * 3 + k:wb + dvi * 3 + k + 1])
                    nc.vector.tensor_add(FW[:], FW[:], YTS[0][:])
                    nc.vector.tensor_add(FW[:], FW[:], YTS[1][:])

                def SE(g):
                    """dot + tree + mask + store for group g."""
                    d = tiles.pop(g)
                    FWF, FWB = d["FWs"][0], d["FWs"][1]
                    nc.vector.tensor_mul(FWF[:], FWF[:], FWB[:])
                    P = FWF[:].rearrange("p a b c -> p (a b) c")
                    wdt = C
                    while wdt > 3:
                        nc.vector.tensor_add(
                            P[:, :, 0:wdt // 2], P[:, :, 0:wdt // 2],
                            P[:, :, wdt // 2:wdt])
                        wdt //= 2
                    ot = yst.tile([128, R], F32, tag="ot")
                    nc.vector.tensor_reduce(
                        ot[:], P[:, :, 0:3], axis=mybir.AxisListType.X,
                        op=OP.add)
                    nc.vector.tensor_mul(ot[:], ot[:], d["mk"][:])
                    nc.sync.dma_start(out=out_d.ap()[g], in_=ot[:])

                # software pipeline: mults of g+1 are emitted around the
                # y-stage/dot of g so no engine queue head-blocks.
                S0(0)
                S0(1)
                SM(0, 0)
                SD(0, 0)
                SM(0, 1)
                SD(0, 1)
                for g in range(GPC):
                    if g + 1 < GPC:
                        SM(g + 1, 0)
                    SBW(g, 0)
                    if g + 1 < GPC:
                        SD(g + 1, 0)
                        SM(g + 1, 1)
                    SBW(g, 1)
                    SE(g)
                    if g + 1 < GPC:
                        SD(g + 1, 1)
                    if g + 2 < GPC:
                        S0(g + 2)

    nc.compile()
    return nc


# ------------------------------------------------------------------ host
def _host_fields(BM, sign, b):
    """Window geometry + separable weights + mask for one warp.
    Mirrors reference f32 math. Returns arrays indexed [h, w]."""
    BMx = BM[b, 0].astype(np.float32)
    BMy = BM[b, 1].astype(np.float32)
    x = np.arange(W, dtype=np.float32)[None, :]
    y = np.arange(H, dtype=np.float32)[:, None]
    s = np.float32(sign)
    ix = (SW * (x[:, :, None] + s * (BMx[:, :, None] + LIN[None, None, :]))
          - np.float32(0.5))
    iy = (SH * (y[:, :, None] + s * (BMy[:, :, None] + LIN[None, None, :]))
          - np.float32(0.5))
    x0f = np.floor(ix)
    y0f = np.floor(iy)
    fx = (ix - x0f).astype(np.float32)
    fy = (iy - y0f).astype(np.float32)
    x0 = x0f.astype(np.int32)
    y0 = y0f.astype(np.int32)

    basex = SW * (x + s * BMx) - np.float32(0.5)
    basey = SH * (y + s * BMy) - np.float32(0.5)
    cx = np.floor(basex + 0.5).astype(np.int32)
    cy = np.floor(basey + 0.5).astype(np.int32)

    xstart = cx - 5 + XPAD
    pair = xstart >> 1
    sL = np.round(s * LIN).astype(np.int32)[None, None, :]
    e_x = x0 - (cx[:, :, None] + sL)
    assert e_x.min() >= -1 and e_x.max() <= 0, (e_x.min(), e_x.max())
    pi = (xstart - 2 * pair)[:, :, None]
    t0 = pi + e_x + 1
    hh, ww, rr = np.meshgrid(np.arange(H), np.arange(W), np.arange(9),
                             indexing="ij")
    qq = rr if sign > 0 else 8 - rr
    wx4 = np.zeros((H, W, 9, 4), np.float32)
    wx4[hh, ww, qq, t0] = 1.0 - fx
    wx4[hh, ww, qq, t0 + 1] = fx

    e_y = y0 - (cy[:, :, None] + sL)
    assert e_y.min() >= -1 and e_y.max() <= 0, (e_y.min(), e_y.max())
    wy3 = np.zeros((H, W, 9, 3), np.float32)
    if sign > 0:
        wy3[hh, ww, rr, e_y + 1] = 1.0 - fy
        wy3[hh, ww, rr, e_y + 2] = fy
        idx0 = (cy - 5 + YPAD) * NPAIR + pair
        idxstep = NPAIR
    else:
        wy3[hh, ww, rr, 1 - e_y] = 1.0 - fy
        wy3[hh, ww, rr, -e_y] = fy
        idx0 = (cy + 5 + YPAD) * NPAIR + pair
        idxstep = -NPAIR
    rlo = idx0 + (10 * idxstep if idxstep < 0 else 0)
    rhi = idx0 + (10 * idxstep if idxstep > 0 else 0)
    assert rlo.min() >= 0 and rhi.max() < NROWS, (rlo.min(), rhi.max())
    assert xstart.min() >= 0 and (2 * pair + 12).max() <= XT

    inbx = ((x0 >= 0) & (x0 <= W - 1)).astype(np.float32)
    inbx1 = ((x0 + 1 >= 0) & (x0 + 1 <= W - 1)).astype(np.float32)
    inby = ((y0 >= 0) & (y0 <= H - 1)).astype(np.float32)
    inby1 = ((y0 + 1 >= 0) & (y0 + 1 <= H - 1)).astype(np.float32)
    mx = (1 - fx) * inbx + fx * inbx1
    my = (1 - fy) * inby + fy * inby1
    m2 = mx[:, :, None, :] * my[:, :, :, None]        # [H, W, dv, du]
    mbin = np.where(m2 < np.float32(0.999), np.float32(0), np.float32(1))
    return dict(wx4=wx4, wy3=wy3, idx0=idx0, idxstep=idxstep, mask=mbin)


def make_in_maps(feature1, feature2, BM):
    f1 = np.ascontiguousarray(np.asarray(feature1, dtype=np.float32))
    f2 = np.ascontiguousarray(np.asarray(feature2, dtype=np.float32))
    bm = np.asarray(BM, dtype=np.float32)

    fields = {}
    for b in range(B):
        fields[(b, +1)] = _host_fields(bm, +1, b)
        fields[(b, -1)] = _host_fields(bm, -1, b)

    in_maps = []
    groups_per_core = []
    for k in range(NCORES):
        gs = list(range(GPC * k, GPC * (k + 1)))
        groups_per_core.append(gs)
        b0 = gs[0] // H
        assert all(g // H == b0 for g in gs)
        wxy = np.zeros((GPC, 128, 126), np.float32)
        msk = np.zeros((GPC, 128, R), np.float32)
        gidx = np.zeros((GPC, 2, 128, 88), np.int16)
        for gi, g in enumerate(gs):
            h = g % H
            for wi, sign in enumerate((+1, -1)):
                fl = fields[(b0, sign)]
                wxy[gi, :, 63 * wi:63 * wi + 36] = \
                    fl["wx4"][h].reshape(128, 36)
                wxy[gi, :, 63 * wi + 36:63 * wi + 63] = \
                    fl["wy3"][h].reshape(128, 27)
                rows = (fl["idx0"][h][None, :]
                        + np.arange(NW)[:, None] * fl["idxstep"])  # [11, 128]
                wrapped = rows.reshape(-1).astype(np.int16)
                wrapped = wrapped.reshape(88, 16).T      # [16, 88]
                gidx[gi, wi] = np.tile(wrapped, (8, 1))
            m = (fields[(b0, +1)]["mask"][h]
                 * fields[(b0, -1)]["mask"][h])          # [W, dv, du]
            msk[gi] = m.reshape(128, R)
        in_maps.append({
            "f2b": f2[b0], "f1b": f1[b0],
            "wxy": wxy, "msk": msk, "gidx": gidx,
        })
    return in_maps, groups_per_core, None


_NC_CACHE = {}


def get_program():
    if "nc" not in _NC_CACHE:
        _NC_CACHE["nc"] = build_program()
    return _NC_CACHE["nc"]


def assemble_output(results, groups_per_core, _unused=None):
    out = np.zeros((B, R, H, W), np.float32)
    for k in range(NCORES):
        core_out = results[k]["out"]          # [GPC, 128, R]
        for gi, g in enumerate(groups_per_core[k]):
            b, h = g // H, g % H
            out[b, :, h, :] = core_out[gi].T
    return out


def kernel(feature1, feature2, BM):
    nc = get_program()
    in_maps, groups_per_core, _ = make_in_maps(feature1, feature2, BM)
    res = bass_utils.run_bass_kernel_spmd(
        nc, in_maps, core_ids=list(range(NCORES)))
    return assemble_output(res.results, groups_per_core)


# revision 23
# speedup vs baseline: 3.0182x; 1.0284x over previous
"""Bass/Tile TRN2 kernel for nn_BilateralCostVolume — windowed-gather design.

out[b,r,h,w] = <bilinear(f2n, p + d_r), bilinear(f1n, p - d_r)> * mask
with d_r = BM + (du,dv), r = dv*9+du, du/dv in linspace(-4,4,9), t=0.5.

Key idea: for each pixel, the 81 displacement samples of one warp all lie in
an 11x11 window around a per-pixel center (BM enters the center; du/dv are
integer-ish shifts).  Gather that window ONCE per (pixel, warp) — 11 rows of
12px x 64c bf16 (1536 B descriptors) from a padded, edge-replicated,
channel-last table — then do separable interpolation shared across
displacements:

  x-stage: XI[du, py, c] = sum_t wx4[du,t] * win[py, du+t, c]   (4 taps)
  y-stage: FW[dv, du, c] = sum_k wy3[dv,k] * XI[du, dv+k, c]    (3 taps)
  dot:     out[r] = sum_c FWF * FWB  (warp B du-axis pre-flipped), * mask

Weights / gather indices / masks are host-precomputed from BM (f32 math
mirroring the reference).  Tables are built on device: l2-normalize over c,
transpose to [y, x, c], pad, cast bf16.

Sharding: 160 (b, h) pixel rows over 8 cores; 128 pixels of a row on SBUF
partitions; each core sees only its batch's features (host slices).
"""

import numpy as np
from ml_dtypes import bfloat16

import concourse.bass as bass
import concourse.bacc as bacc
import concourse.mybir as mybir
import concourse.tile as tile
from concourse import bass_utils
from concourse.masks import make_identity

MD = 4
R = 81
B, C, H, W = 2, 48, 80, 128
SW = np.float32(W) / np.float32(W - 1)
SH = np.float32(H) / np.float32(H - 1)
CP = 64                  # padded channels in tables
XPAD = 12
YPAD = 12
XT = W + 2 * XPAD + 2    # 154 (even)
YT = H + 2 * YPAD        # 104
NPAIR = XT // 2          # 77
NROWS = YT * NPAIR       # 8008
NROWS_AL = 8016          # allocated rows (tail pad for 768-elem overrun)
NCORES = 8
GPC = 20                 # (b, h) groups per core
NW = 11                  # window rows per pixel
EL = 768                 # gather elem_size (12 px * 64 c, bf16 -> 1536 B)
ES = 128                 # gather elem_step (2 px * 64 c = 256 B)
NIDX = NW * 128          # 1408 idxs per gather

F32 = mybir.dt.float32
I16 = mybir.dt.int16
BF16 = mybir.dt.bfloat16
AF = mybir.ActivationFunctionType
OP = mybir.AluOpType

LIN = np.linspace(-MD, MD, 2 * MD + 1).astype(np.float32)


# ------------------------------------------------------------------ program
def _overlap_ap(t_ap, offset_elems=0):
    """View a [NROWS_AL, ES] dram tile as overlapped gather rows
    [[ES, NROWS_AL], [1, EL]]."""
    a = t_ap.copy()
    v = a.ap
    v.clear()
    v.extend([(ES, NROWS_AL - 5), (1, EL)])
    a.offset = a.offset + offset_elems
    return a


def build_program(dbg=False):
    nc = bacc.Bacc(
        "TRN2",
        target_bir_lowering=False,
        debug=False,
        enable_asserts=False,
        num_devices=NCORES,
        num_swdge_queues=2,
    )

    f2b_d = nc.dram_tensor("f2b", [C, H, W], F32, kind="ExternalInput")
    f1b_d = nc.dram_tensor("f1b", [C, H, W], F32, kind="ExternalInput")
    wxy_d = nc.dram_tensor("wxy", [GPC, 128, 126], F32, kind="ExternalInput")
    msk_d = nc.dram_tensor("msk", [GPC, 128, R], F32, kind="ExternalInput")
    gidx_d = nc.dram_tensor("gidx", [GPC, 2, 128, 88], I16,
                            kind="ExternalInput")
    out_d = nc.dram_tensor("out", [GPC, 128, R], F32, kind="ExternalOutput")

    with tile.TileContext(nc) as tc:
        with (
            tc.tile_pool(name="const", bufs=1) as constp,
            tc.tile_pool(name="dram", bufs=1, space="DRAM") as dramp,
        ):
            ident = constp.tile([128, 128], F32)
            make_identity(nc, ident[:])

            tabF = dramp.tile([NROWS_AL, ES], BF16)   # f2n table (warp F)
            tabB = dramp.tile([NROWS_AL, ES], BF16)   # f1n table (warp B)

            # ---------------- Phase 1: normalize + padded tables ----------
            # single pool + per-plane tags so the two planes pipeline
            with (
                tc.tile_pool(name="p1", bufs=1) as p1,
                tc.tile_pool(name="ps1", bufs=2, space="PSUM") as ps1,
            ):
                for pi_, (src_d, tab) in enumerate(
                        ((f2b_d, tabF), (f1b_d, tabB))):
                    fc = p1.tile([C, H * W], F32, tag=f"fc{pi_}")
                    nc.sync.dma_start(out=fc[:], in_=src_d.ap().rearrange(
                        "c h w -> c (h w)"))
                    T = p1.tile([128, H, C], F32, tag=f"T{pi_}")
                    for hb in range(8):
                        pt = ps1.tile([128, 10 * C], F32, tag=f"pt{pi_}")
                        for j in range(10):
                            h = hb * 10 + j
                            nc.tensor.transpose(
                                out=pt[:, j * C:(j + 1) * C],
                                in_=fc[:, h * W:(h + 1) * W],
                                identity=ident[:C, :C])
                        nc.scalar.copy(
                            T[:, hb * 10:(hb + 1) * 10, :], pt[:])
                    sq = p1.tile([128, H, C], F32, tag=f"sq{pi_}")
                    nc.scalar.square(sq[:], T[:])
                    ssq = p1.tile([128, H], F32, tag=f"ssq{pi_}")
                    nc.vector.tensor_reduce(
                        ssq[:], sq[:], axis=mybir.AxisListType.X, op=OP.add)
                    rn = p1.tile([128, H], F32, tag=f"rn{pi_}")
                    nc.scalar.activation(rn[:], ssq[:], AF.Sqrt,
                                         bias=np.float32(1e-6))
                    nc.vector.reciprocal(rn[:], rn[:])
                    re = p1.tile([128, H, C], BF16, tag=f"re{pi_}")
                    nc.vector.tensor_copy(
                        re[:], rn[:].unsqueeze(-1).broadcast_to([128, H, C]))
                    tb = p1.tile([128, H, C], BF16, tag=f"tb{pi_}")
                    nc.vector.tensor_copy(tb[:], T[:])
                    tn = p1.tile([128, H, C], BF16, tag=f"tn{pi_}")
                    nc.vector.tensor_mul(tn[:], tb[:], re[:])

                    # interior write: px (XPAD + w) of row (YPAD + h)
                    dst = tab[:].copy()
                    v = dst.ap
                    v.clear()
                    # dims: (w 128 part-ish? no — DMA from SBUF [128,...]
                    # source partitions = w); dst elem offset:
                    # ((YPAD+h)*XT + XPAD + w)*CP + c
                    v.extend([(CP, 128), (XT * CP, H), (1, C)])
                    dst.offset = dst.offset + (YPAD * XT + XPAD) * CP
                    nc.sync.dma_start(out=dst, in_=tn[:])

                    # x pads: left cols [0, XPAD) <- col x=0 ; right
                    # [XPAD+W, XT) <- col x=W-1  (dram->dram, bcast px)
                    for px0, npx, srcx in ((0, XPAD, 0),
                                           (XPAD + W, XT - XPAD - W, W - 1)):
                        sap = tab[:].copy()
                        v = sap.ap
                        v.clear()
                        v.extend([(XT * CP, YT - 24), (0, npx), (1, CP)])
                        sap.offset = (sap.offset
                                      + (YPAD * XT + XPAD + srcx) * CP)
                        dap = tab[:].copy()
                        v = dap.ap
                        v.clear()
                        v.extend([(XT * CP, YT - 24), (CP, npx), (1, CP)])
                        dap.offset = dap.offset + (YPAD * XT + px0) * CP
                        nc.sync.dma_start(out=dap, in_=sap)

                    # y pads: rows [0, YPAD) <- row y=0 ; [YPAD+H, YT) <- last
                    for y0, ny, srcy in ((0, YPAD, YPAD),
                                         (YPAD + H, YT - YPAD - H,
                                          YPAD + H - 1)):
                        sap = tab[:].copy()
                        v = sap.ap
                        v.clear()
                        v.extend([(0, ny), (1, XT * CP)])
                        sap.offset = sap.offset + srcy * XT * CP
                        dap = tab[:].copy()
                        v = dap.ap
                        v.clear()
                        v.extend([(XT * CP, ny), (1, XT * CP)])
                        dap.offset = dap.offset + y0 * XT * CP
                        nc.sync.dma_start(out=dap, in_=sap)

            # ---------------- Phase 2: per-group windows ------------------
            # Software-pipelined emission: loads/gathers of group g+2 and
            # mults of group g+1 are emitted before the adds/dot of group g
            # so in-order engine queues never stall on cross-engine deps.
            with (
                tc.tile_pool(name="win", bufs=2) as winp,
                tc.tile_pool(name="ld", bufs=3) as ldp,
                tc.tile_pool(name="cmp", bufs=2) as cmp_,
                tc.tile_pool(name="yst", bufs=1) as yst,
            ):
                XM_ENG = {(0, 0): "v", (0, 1): "a", (0, 2): "a",
                          (0, 3): "a", (1, 0): "a", (1, 1): "p",
                          (1, 2): "p", (1, 3): "v"}
                YM_ENG = {(0, 0): "v", (0, 1): "v", (0, 2): "a",
                          (1, 0): "a", (1, 1): "p", (1, 2): "p"}

                def mul_op(eng, out, in0, sc):
                    if eng == "a":
                        nc.scalar.mul(out, in0, sc)
                    elif eng == "p":
                        nc.gpsimd.tensor_scalar(
                            out=out, in0=in0, scalar1=sc, scalar2=None,
                            op0=OP.mult)
                    else:
                        nc.vector.tensor_scalar(
                            out=out, in0=in0, scalar1=sc, scalar2=None,
                            op0=OP.mult)

                tiles = {}

                def S0(g):
                    d = {}
                    d["wv"] = ldp.tile([128, 126], F32, tag="wv", name="wv")
                    nc.sync.dma_start(out=d["wv"][:], in_=wxy_d.ap()[g])
                    d["mk"] = ldp.tile([128, R], F32, tag="mk", name="mk")
                    nc.sync.dma_start(out=d["mk"][:], in_=msk_d.ap()[g])
                    d["wins"] = []
                    for wi, tab in enumerate((tabF, tabB)):
                        gx = ldp.tile([128, 88], I16, tag=f"gx{wi}",
                                      name=f"gx{wi}")
                        nc.sync.dma_start(out=gx[:], in_=gidx_d.ap()[g, wi])
                        win = winp.tile([128, NW, EL], BF16, tag=f"win{wi}",
                                        name=f"win{wi}")
                        nc.gpsimd.dma_gather(
                            out_ap=win[:],
                            in_ap=_overlap_ap(tab[:]),
                            idxs_ap=gx[:],
                            num_idxs=NIDX,
                            num_idxs_reg=NIDX,
                            elem_size=EL,
                            elem_step=ES,
                            single_packet=False,
                            queue_num=wi,
                        )
                        d["wins"].append(win)
                    d["TMPS"] = {}
                    d["XIs"] = {}
                    d["FWs"] = {}
                    tiles[g] = d

                def SM(g, wi):
                    """Tap-product multiplies for warp wi of group g."""
                    d = tiles[g]
                    wv = d["wv"]
                    win = d["wins"][wi]
                    TMPS = [cmp_.tile([128, 9, NW, C], BF16, tag=f"TMP{j}",
                                      name=f"TMP{j}") for j in range(3)]
                    d["TMPS"][wi] = TMPS
                    XI = cmp_.tile([128, 9, NW, C], BF16, tag=f"XI{wi}",
                                   name=f"XI{wi}")
                    d["XIs"][wi] = XI
                    wb = 63 * wi
                    for t in range(4):
                        dstt = XI if t == 0 else TMPS[t - 1]
                        eng = XM_ENG[(wi, t)]
                        for dui in range(9):
                            do = dui if wi == 0 else 8 - dui
                            mul_op(
                                eng, dstt[:, do, :, :],
                                win[:, :, (dui + t) * CP:(dui + t) * CP + C],
                                wv[:, wb + dui * 4 + t:wb + dui * 4 + t + 1])

                def SD(g, wi):
                    """x-adds for warp wi of group g (DVE)."""
                    d = tiles[g]
                    XI = d["XIs"][wi]
                    TMPS = d["TMPS"][wi]
                    nc.vector.tensor_add(TMPS[0][:], TMPS[0][:], TMPS[1][:])
                    nc.vector.tensor_add(XI[:], XI[:], TMPS[2][:])
                    nc.vector.tensor_add(XI[:], XI[:], TMPS[0][:])

                def SBW(g, wi):
                    """y-stage for warp wi of group g."""
                    d = tiles[g]
                    wv = d["wv"]
                    XI = d["XIs"][wi]
                    YTS = [yst.tile([128, 9, 9, C], BF16, tag=f"YT{j}",
                                    name=f"YT{j}") for j in range(2)]
                    FW = yst.tile([128, 9, 9, C], BF16, tag=f"FW{wi}",
                                  name=f"FW{wi}")
                    d["FWs"][wi] = FW
                    wb = 63 * wi + 36
                    for k in range(3):
                        dstt = FW if k == 0 else YTS[k - 1]
                        eng = YM_ENG[(wi, k)]
                        for dvi in range(9):
                            mul_op(
                                eng, dstt[:, dvi, :, :],
                                XI[:, :, dvi + k, :],
                                wv[:, wb + dvi * 3 + k:wb + dvi * 3 + k + 1])
                    nc.vector.tensor_add(FW[:], FW[:], YTS[0][:])
                    nc.vector.tensor_add(FW[:], FW[:], YTS[1][:])

                def SE(g):
                    """dot + tree + mask + store for group g."""
                    d = tiles.pop(g)
                    FWF, FWB = d["FWs"][0], d["FWs"][1]
                    nc.vector.tensor_mul(FWF[:], FWF[:], FWB[:])
                    P = FWF[:].rearrange("p a b c -> p (a b) c")
                    wdt = C
                    while wdt > 3:
                        nc.vector.tensor_add(
                            P[:, :, 0:wdt // 2], P[:, :, 0:wdt // 2],
                            P[:, :, wdt // 2:wdt])
                        wdt //= 2
                    ot = yst.tile([128, R], F32, tag="ot")
                    nc.vector.tensor_reduce(
                        ot[:], P[:, :, 0:3], axis=mybir.AxisListType.X,
                        op=OP.add)
                    nc.vector.tensor_mul(ot[:], ot[:], d["mk"][:])
                    nc.sync.dma_start(out=out_d.ap()[g], in_=ot[:])

                # software pipeline: mults of g+1 are emitted around the
                # y-stage/dot of g so no engine queue head-blocks.
                S0(0)
                S0(1)
                SM(0, 0)
                SD(0, 0)
                SM(0, 1)
                SD(0, 1)
                for g in range(GPC):
                    if g + 1 < GPC:
                        SM(g + 1, 0)
                    SBW(g, 0)
                    if g + 1 < GPC:
                        SD(g + 1, 0)
                        SM(g + 1, 1)
                    SBW(g, 1)
                    SE(g)
                    if g + 1 < GPC:
                        SD(g + 1, 1)
                    if g + 2 < GPC:
                        S0(g + 2)

    nc.compile()
    return nc


# ------------------------------------------------------------------ host
def _host_fields(BM, sign, b):
    """Window geometry + separable weights + mask for one warp.
    Mirrors reference f32 math. Returns arrays indexed [h, w]."""
    BMx = BM[b, 0].astype(np.float32)
    BMy = BM[b, 1].astype(np.float32)
    x = np.arange(W, dtype=np.float32)[None, :]
    y = np.arange(H, dtype=np.float32)[:, None]
    s = np.float32(sign)
    ix = (SW * (x[:, :, None] + s * (BMx[:, :, None] + LIN[None, None, :]))
          - np.float32(0.5))
    iy = (SH * (y[:, :, None] + s * (BMy[:, :, None] + LIN[None, None, :]))
          - np.float32(0.5))
    x0f = np.floor(ix)
    y0f = np.floor(iy)
    fx = (ix - x0f).astype(np.float32)
    fy = (iy - y0f).astype(np.float32)
    x0 = x0f.astype(np.int32)
    y0 = y0f.astype(np.int32)

    basex = SW * (x + s * BMx) - np.float32(0.5)
    basey = SH * (y + s * BMy) - np.float32(0.5)
    cx = np.floor(basex + 0.5).astype(np.int32)
    cy = np.floor(basey + 0.5).astype(np.int32)

    xstart = cx - 5 + XPAD
    pair = xstart >> 1
    sL = np.round(s * LIN).astype(np.int32)[None, None, :]
    e_x = x0 - (cx[:, :, None] + sL)
    assert e_x.min() >= -1 and e_x.max() <= 0, (e_x.min(), e_x.max())
    pi = (xstart - 2 * pair)[:, :, None]
    t0 = pi + e_x + 1
    hh, ww, rr = np.meshgrid(np.arange(H), np.arange(W), np.arange(9),
                             indexing="ij")
    qq = rr if sign > 0 else 8 - rr
    wx4 = np.zeros((H, W, 9, 4), np.float32)
    wx4[hh, ww, qq, t0] = 1.0 - fx
    wx4[hh, ww, qq, t0 + 1] = fx

    e_y = y0 - (cy[:, :, None] + sL)
    assert e_y.min() >= -1 and e_y.max() <= 0, (e_y.min(), e_y.max())
    wy3 = np.zeros((H, W, 9, 3), np.float32)
    if sign > 0:
        wy3[hh, ww, rr, e_y + 1] = 1.0 - fy
        wy3[hh, ww, rr, e_y + 2] = fy
        idx0 = (cy - 5 + YPAD) * NPAIR + pair
        idxstep = NPAIR
    else:
        wy3[hh, ww, rr, 1 - e_y] = 1.0 - fy
        wy3[hh, ww, rr, -e_y] = fy
        idx0 = (cy + 5 + YPAD) * NPAIR + pair
        idxstep = -NPAIR
    rlo = idx0 + (10 * idxstep if idxstep < 0 else 0)
    rhi = idx0 + (10 * idxstep if idxstep > 0 else 0)
    assert rlo.min() >= 0 and rhi.max() < NROWS, (rlo.min(), rhi.max())
    assert xstart.min() >= 0 and (2 * pair + 12).max() <= XT

    inbx = ((x0 >= 0) & (x0 <= W - 1)).astype(np.float32)
    inbx1 = ((x0 + 1 >= 0) & (x0 + 1 <= W - 1)).astype(np.float32)
    inby = ((y0 >= 0) & (y0 <= H - 1)).astype(np.float32)
    inby1 = ((y0 + 1 >= 0) & (y0 + 1 <= H - 1)).astype(np.float32)
    mx = (1 - fx) * inbx + fx * inbx1
    my = (1 - fy) * inby + fy * inby1
    m2 = mx[:, :, None, :] * my[:, :, :, None]        # [H, W, dv, du]
    mbin = np.where(m2 < np.float32(0.999), np.float32(0), np.float32(1))
    return dict(wx4=wx4, wy3=wy3, idx0=idx0, idxstep=idxstep, mask=mbin)


def make_in_maps(feature1, feature2, BM):
    f1 = np.ascontiguousarray(np.asarray(feature1, dtype=np.float32))
    f2 = np.ascontiguousarray(np.asarray(feature2, dtype=np.float32))
    bm = np.asarray(BM, dtype=np.float32)

    fields = {}
    for b in range(B):
        fields[(b, +1)] = _host_fields(bm, +1, b)
        fields[(b, -1)] = _host_fields(bm, -1, b)

    in_maps = []
    groups_per_core = []
    for k in range(NCORES):
        gs = list(range(GPC * k, GPC * (k + 1)))
        groups_per_core.append(gs)
        b0 = gs[0] // H
        assert all(g // H == b0 for g in gs)
        wxy = np.zeros((GPC, 128, 126), np.float32)
        msk = np.zeros((GPC, 128, R), np.float32)
        gidx = np.zeros((GPC, 2, 128, 88), np.int16)
        for gi, g in enumerate(gs):
            h = g % H
            for wi, sign in enumerate((+1, -1)):
                fl = fields[(b0, sign)]
                wxy[gi, :, 63 * wi:63 * wi + 36] = \
                    fl["wx4"][h].reshape(128, 36)
                wxy[gi, :, 63 * wi + 36:63 * wi + 63] = \
                    fl["wy3"][h].reshape(128, 27)
                rows = (fl["idx0"][h][None, :]
                        + np.arange(NW)[:, None] * fl["idxstep"])  # [11, 128]
                wrapped = rows.reshape(-1).astype(np.int16)
                wrapped = wrapped.reshape(88, 16).T      # [16, 88]
                gidx[gi, wi] = np.tile(wrapped, (8, 1))
            m = (fields[(b0, +1)]["mask"][h]
                 * fields[(b0, -1)]["mask"][h])          # [W, dv, du]
            msk[gi] = m.reshape(128, R)
        in_maps.append({
            "f2b": f2[b0], "f1b": f1[b0],
            "wxy": wxy, "msk": msk, "gidx": gidx,
        })
    return in_maps, groups_per_core, None


_NC_CACHE = {}


def get_program():
    if "nc" not in _NC_CACHE:
        _NC_CACHE["nc"] = build_program()
    return _NC_CACHE["nc"]


def assemble_output(results, groups_per_core, _unused=None):
    out = np.zeros((B, R, H, W), np.float32)
    for k in range(NCORES):
        core_out = results[k]["out"]          # [GPC, 128, R]
        for gi, g in enumerate(groups_per_core[k]):
            b, h = g // H, g % H
            out[b, :, h, :] = core_out[gi].T
    return out


def kernel(feature1, feature2, BM):
    nc = get_program()
    in_maps, groups_per_core, _ = make_in_maps(feature1, feature2, BM)
    res = bass_utils.run_bass_kernel_spmd(
        nc, in_maps, core_ids=list(range(NCORES)))
    return assemble_output(res.results, groups_per_core)


# revision 27
# speedup vs baseline: 3.0252x; 1.0023x over previous
"""Bass/Tile TRN2 kernel for nn_BilateralCostVolume — windowed-gather design.

out[b,r,h,w] = <bilinear(f2n, p + d_r), bilinear(f1n, p - d_r)> * mask
with d_r = BM + (du,dv), r = dv*9+du, du/dv in linspace(-4,4,9), t=0.5.

Key idea: for each pixel, the 81 displacement samples of one warp all lie in
an 11x11 window around a per-pixel center (BM enters the center; du/dv are
integer-ish shifts).  Gather that window ONCE per (pixel, warp) — 11 rows of
12px x 64c bf16 (1536 B descriptors) from a padded, edge-replicated,
channel-last table — then do separable interpolation shared across
displacements:

  x-stage: XI[du, py, c] = sum_t wx4[du,t] * win[py, du+t, c]   (4 taps)
  y-stage: FW[dv, du, c] = sum_k wy3[dv,k] * XI[du, dv+k, c]    (3 taps)
  dot:     out[r] = sum_c FWF * FWB  (warp B du-axis pre-flipped), * mask

Weights / gather indices / masks are host-precomputed from BM (f32 math
mirroring the reference).  Tables are built on device: l2-normalize over c,
transpose to [y, x, c], pad, cast bf16.

Sharding: 160 (b, h) pixel rows over 8 cores; 128 pixels of a row on SBUF
partitions; each core sees only its batch's features (host slices).
"""

import numpy as np
from ml_dtypes import bfloat16

import concourse.bass as bass
import concourse.bacc as bacc
import concourse.mybir as mybir
import concourse.tile as tile
from concourse import bass_utils
from concourse.masks import make_identity

MD = 4
R = 81
B, C, H, W = 2, 48, 80, 128
SW = np.float32(W) / np.float32(W - 1)
SH = np.float32(H) / np.float32(H - 1)
CP = 64                  # padded channels in tables
XPAD = 12
YPAD = 12
XT = W + 2 * XPAD + 2    # 154 (even)
YT = H + 2 * YPAD        # 104
NPAIR = XT // 2          # 77
NROWS = YT * NPAIR       # 8008
NROWS_AL = 8016          # allocated rows (tail pad for 768-elem overrun)
NCORES = 8
GPC = 20                 # (b, h) groups per core
NW = 11                  # window rows per pixel
EL = 768                 # gather elem_size (12 px * 64 c, bf16 -> 1536 B)
ES = 128                 # gather elem_step (2 px * 64 c = 256 B)
NIDX = NW * 128          # 1408 idxs per gather

F32 = mybir.dt.float32
I16 = mybir.dt.int16
BF16 = mybir.dt.bfloat16
AF = mybir.ActivationFunctionType
OP = mybir.AluOpType

LIN = np.linspace(-MD, MD, 2 * MD + 1).astype(np.float32)


# ------------------------------------------------------------------ program
def _overlap_ap(t_ap, offset_elems=0):
    """View a [NROWS_AL, ES] dram tile as overlapped gather rows
    [[ES, NROWS_AL], [1, EL]]."""
    a = t_ap.copy()
    v = a.ap
    v.clear()
    v.extend([(ES, NROWS_AL - 5), (1, EL)])
    a.offset = a.offset + offset_elems
    return a


def build_program(dbg=False):
    nc = bacc.Bacc(
        "TRN2",
        target_bir_lowering=False,
        debug=False,
        enable_asserts=False,
        num_devices=NCORES,
        num_swdge_queues=2,
    )

    f2b_d = nc.dram_tensor("f2b", [C, H, W], F32, kind="ExternalInput")
    f1b_d = nc.dram_tensor("f1b", [C, H, W], F32, kind="ExternalInput")
    wxy_d = nc.dram_tensor("wxy", [GPC, 128, 126], F32, kind="ExternalInput")
    msk_d = nc.dram_tensor("msk", [GPC, 128, R], F32, kind="ExternalInput")
    gidx_d = nc.dram_tensor("gidx", [GPC, 2, 128, 88], I16,
                            kind="ExternalInput")
    out_d = nc.dram_tensor("out", [GPC, 128, R], F32, kind="ExternalOutput")

    with tile.TileContext(nc) as tc:
        with (
            tc.tile_pool(name="const", bufs=1) as constp,
            tc.tile_pool(name="dram", bufs=1, space="DRAM") as dramp,
        ):
            ident = constp.tile([128, 128], F32)
            make_identity(nc, ident[:])

            tabF = dramp.tile([NROWS_AL, ES], BF16)   # f2n table (warp F)
            tabB = dramp.tile([NROWS_AL, ES], BF16)   # f1n table (warp B)

            # ---------------- Phase 1: normalize + padded tables ----------
            # single pool + per-plane tags so the two planes pipeline
            with (
                tc.tile_pool(name="p1", bufs=1) as p1,
                tc.tile_pool(name="ps1", bufs=2, space="PSUM") as ps1,
            ):
                for pi_, (src_d, tab) in enumerate(
                        ((f2b_d, tabF), (f1b_d, tabB))):
                    fc = p1.tile([C, H * W], F32, tag=f"fc{pi_}")
                    nc.sync.dma_start(out=fc[:], in_=src_d.ap().rearrange(
                        "c h w -> c (h w)"))
                    T = p1.tile([128, H, C], F32, tag=f"T{pi_}")
                    for hb in range(8):
                        pt = ps1.tile([128, 10 * C], F32, tag=f"pt{pi_}")
                        for j in range(10):
                            h = hb * 10 + j
                            nc.tensor.transpose(
                                out=pt[:, j * C:(j + 1) * C],
                                in_=fc[:, h * W:(h + 1) * W],
                                identity=ident[:C, :C])
                        nc.scalar.copy(
                            T[:, hb * 10:(hb + 1) * 10, :], pt[:])
                    sq = p1.tile([128, H, C], F32, tag=f"sq{pi_}")
                    nc.scalar.square(sq[:], T[:])
                    ssq = p1.tile([128, H], F32, tag=f"ssq{pi_}")
                    nc.vector.tensor_reduce(
                        ssq[:], sq[:], axis=mybir.AxisListType.X, op=OP.add)
                    rn = p1.tile([128, H], F32, tag=f"rn{pi_}")
                    nc.scalar.activation(rn[:], ssq[:], AF.Sqrt,
                                         bias=np.float32(1e-6))
                    nc.vector.reciprocal(rn[:], rn[:])
                    re = p1.tile([128, H, C], BF16, tag=f"re{pi_}")
                    nc.vector.tensor_copy(
                        re[:], rn[:].unsqueeze(-1).broadcast_to([128, H, C]))
                    tb = p1.tile([128, H, C], BF16, tag=f"tb{pi_}")
                    nc.vector.tensor_copy(tb[:], T[:])
                    tn = p1.tile([128, H, C], BF16, tag=f"tn{pi_}")
                    nc.vector.tensor_mul(tn[:], tb[:], re[:])

                    # interior write: px (XPAD + w) of row (YPAD + h)
                    dst = tab[:].copy()
                    v = dst.ap
                    v.clear()
                    # dims: (w 128 part-ish? no — DMA from SBUF [128,...]
                    # source partitions = w); dst elem offset:
                    # ((YPAD+h)*XT + XPAD + w)*CP + c
                    v.extend([(CP, 128), (XT * CP, H), (1, C)])
                    dst.offset = dst.offset + (YPAD * XT + XPAD) * CP
                    nc.sync.dma_start(out=dst, in_=tn[:])

                    # x pads: left cols [0, XPAD) <- col x=0 ; right
                    # [XPAD+W, XT) <- col x=W-1  (dram->dram, bcast px)
                    for px0, npx, srcx in ((0, XPAD, 0),
                                           (XPAD + W, XT - XPAD - W, W - 1)):
                        sap = tab[:].copy()
                        v = sap.ap
                        v.clear()
                        v.extend([(XT * CP, YT - 24), (0, npx), (1, CP)])
                        sap.offset = (sap.offset
                                      + (YPAD * XT + XPAD + srcx) * CP)
                        dap = tab[:].copy()
                        v = dap.ap
                        v.clear()
                        v.extend([(XT * CP, YT - 24), (CP, npx), (1, CP)])
                        dap.offset = dap.offset + (YPAD * XT + px0) * CP
                        nc.sync.dma_start(out=dap, in_=sap)

                    # y pads: rows [0, YPAD) <- row y=0 ; [YPAD+H, YT) <- last
                    for y0, ny, srcy in ((0, YPAD, YPAD),
                                         (YPAD + H, YT - YPAD - H,
                                          YPAD + H - 1)):
                        sap = tab[:].copy()
                        v = sap.ap
                        v.clear()
                        v.extend([(0, ny), (1, XT * CP)])
                        sap.offset = sap.offset + srcy * XT * CP
                        dap = tab[:].copy()
                        v = dap.ap
                        v.clear()
                        v.extend([(XT * CP, ny), (1, XT * CP)])
                        dap.offset = dap.offset + y0 * XT * CP
                        nc.sync.dma_start(out=dap, in_=sap)

            # ---------------- Phase 2: per-group windows ------------------
            # Software-pipelined emission: loads/gathers of group g+2 and
            # mults of group g+1 are emitted before the adds/dot of group g
            # so in-order engine queues never stall on cross-engine deps.
            with (
                tc.tile_pool(name="win", bufs=2) as winp,
                tc.tile_pool(name="ld", bufs=4) as ldp,
                tc.tile_pool(name="cmp", bufs=2) as cmp_,
                tc.tile_pool(name="yst", bufs=1) as yst,
            ):
                XM_ENG = {(0, 0): "v", (0, 1): "a", (0, 2): "a",
                          (0, 3): "a", (1, 0): "a", (1, 1): "p",
                          (1, 2): "p", (1, 3): "v"}
                YM_ENG = {(0, 0): "v", (0, 1): "v", (0, 2): "a",
                          (1, 0): "a", (1, 1): "p", (1, 2): "p"}

                def mul_op(eng, out, in0, sc):
                    if eng == "a":
                        nc.scalar.mul(out, in0, sc)
                    elif eng == "p":
                        nc.gpsimd.tensor_scalar(
                            out=out, in0=in0, scalar1=sc, scalar2=None,
                            op0=OP.mult)
                    else:
                        nc.vector.tensor_scalar(
                            out=out, in0=in0, scalar1=sc, scalar2=None,
                            op0=OP.mult)

                tiles = {}

                def S0(g):
                    d = {}
                    d["wv"] = ldp.tile([128, 126], F32, tag="wv", name="wv")
                    nc.sync.dma_start(out=d["wv"][:], in_=wxy_d.ap()[g])
                    d["mk"] = ldp.tile([128, R], F32, tag="mk", name="mk")
                    nc.sync.dma_start(out=d["mk"][:], in_=msk_d.ap()[g])
                    d["wins"] = []
                    for wi, tab in enumerate((tabF, tabB)):
                        gx = ldp.tile([128, 88], I16, tag=f"gx{wi}",
                                      name=f"gx{wi}")
                        nc.sync.dma_start(out=gx[:], in_=gidx_d.ap()[g, wi])
                        win = winp.tile([128, NW, EL], BF16, tag=f"win{wi}",
                                        name=f"win{wi}")
                        nc.gpsimd.dma_gather(
                            out_ap=win[:],
                            in_ap=_overlap_ap(tab[:]),
                            idxs_ap=gx[:],
                            num_idxs=NIDX,
                            num_idxs_reg=NIDX,
                            elem_size=EL,
                            elem_step=ES,
                            single_packet=False,
                            queue_num=wi,
                        )
                        d["wins"].append(win)
                    d["TMPS"] = {}
                    d["XIs"] = {}
                    d["FWs"] = {}
                    tiles[g] = d

                def SM(g, wi):
                    """Tap-product multiplies for warp wi of group g."""
                    d = tiles[g]
                    wv = d["wv"]
                    win = d["wins"][wi]
                    TMPS = [cmp_.tile([128, 9, NW, C], BF16, tag=f"TMP{j}",
                                      name=f"TMP{j}") for j in range(3)]
                    d["TMPS"][wi] = TMPS
                    XI = cmp_.tile([128, 9, NW, C], BF16, tag=f"XI{wi}",
                                   name=f"XI{wi}")
                    d["XIs"][wi] = XI
                    wb = 63 * wi
                    for t in range(4):
                        dstt = XI if t == 0 else TMPS[t - 1]
                        eng = XM_ENG[(wi, t)]
                        for dui in range(9):
                            do = dui if wi == 0 else 8 - dui
                            mul_op(
                                eng, dstt[:, do, :, :],
                                win[:, :, (dui + t) * CP:(dui + t) * CP + C],
                                wv[:, wb + dui * 4 + t:wb + dui * 4 + t + 1])

                def SD(g, wi):
                    """x-adds for warp wi of group g (DVE)."""
                    d = tiles[g]
                    XI = d["XIs"][wi]
                    TMPS = d["TMPS"][wi]
                    nc.vector.tensor_add(TMPS[0][:], TMPS[0][:], TMPS[1][:])
                    nc.vector.tensor_add(XI[:], XI[:], TMPS[2][:])
                    nc.vector.tensor_add(XI[:], XI[:], TMPS[0][:])

                def SBW(g, wi):
                    """y-stage for warp wi of group g."""
                    d = tiles[g]
                    wv = d["wv"]
                    XI = d["XIs"][wi]
                    YTS = [yst.tile([128, 9, 9, C], BF16, tag=f"YT{j}",
                                    name=f"YT{j}") for j in range(2)]
                    FW = yst.tile([128, 9, 9, C], BF16, tag=f"FW{wi}",
                                  name=f"FW{wi}")
                    d["FWs"][wi] = FW
                    wb = 63 * wi + 36
                    for k in range(3):
                        dstt = FW if k == 0 else YTS[k - 1]
                        eng = YM_ENG[(wi, k)]
                        for dvi in range(9):
                            mul_op(
                                eng, dstt[:, dvi, :, :],
                                XI[:, :, dvi + k, :],
                                wv[:, wb + dvi * 3 + k:wb + dvi * 3 + k + 1])
                    nc.vector.tensor_add(FW[:], FW[:], YTS[0][:])
                    nc.vector.tensor_add(FW[:], FW[:], YTS[1][:])

                def SE(g):
                    """dot + tree + mask + store for group g."""
                    d = tiles.pop(g)
                    FWF, FWB = d["FWs"][0], d["FWs"][1]
                    nc.vector.tensor_mul(FWF[:], FWF[:], FWB[:])
                    P = FWF[:].rearrange("p a b c -> p (a b) c")
                    wdt = C
                    while wdt > 3:
                        nc.vector.tensor_add(
                            P[:, :, 0:wdt // 2], P[:, :, 0:wdt // 2],
                            P[:, :, wdt // 2:wdt])
                        wdt //= 2
                    ot = yst.tile([128, R], F32, tag="ot")
                    nc.vector.tensor_reduce(
                        ot[:], P[:, :, 0:3], axis=mybir.AxisListType.X,
                        op=OP.add)
                    nc.gpsimd.tensor_mul(ot[:], ot[:], d["mk"][:])
                    nc.sync.dma_start(out=out_d.ap()[g], in_=ot[:])

                # software pipeline: mults of g+1 are emitted around the
                # y-stage/dot of g so no engine queue head-blocks.
                S0(0)
                S0(1)
                SM(0, 0)
                SD(0, 0)
                SM(0, 1)
                SD(0, 1)
                for g in range(GPC):
                    if g + 1 < GPC:
                        SM(g + 1, 0)
                    SBW(g, 0)
                    if g + 1 < GPC:
                        SD(g + 1, 0)
                        SM(g + 1, 1)
                    SBW(g, 1)
                    SE(g)
                    if g + 1 < GPC:
                        SD(g + 1, 1)
                    if g + 2 < GPC:
                        S0(g + 2)

    nc.compile()
    return nc


# ------------------------------------------------------------------ host
def _host_fields(BM, sign, b):
    """Window geometry + separable weights + mask for one warp.
    Mirrors reference f32 math. Returns arrays indexed [h, w]."""
    BMx = BM[b, 0].astype(np.float32)
    BMy = BM[b, 1].astype(np.float32)
    x = np.arange(W, dtype=np.float32)[None, :]
    y = np.arange(H, dtype=np.float32)[:, None]
    s = np.float32(sign)
    ix = (SW * (x[:, :, None] + s * (BMx[:, :, None] + LIN[None, None, :]))
          - np.float32(0.5))
    iy = (SH * (y[:, :, None] + s * (BMy[:, :, None] + LIN[None, None, :]))
          - np.float32(0.5))
    x0f = np.floor(ix)
    y0f = np.floor(iy)
    fx = (ix - x0f).astype(np.float32)
    fy = (iy - y0f).astype(np.float32)
    x0 = x0f.astype(np.int32)
    y0 = y0f.astype(np.int32)

    basex = SW * (x + s * BMx) - np.float32(0.5)
    basey = SH * (y + s * BMy) - np.float32(0.5)
    cx = np.floor(basex + 0.5).astype(np.int32)
    cy = np.floor(basey + 0.5).astype(np.int32)

    xstart = cx - 5 + XPAD
    pair = xstart >> 1
    sL = np.round(s * LIN).astype(np.int32)[None, None, :]
    e_x = x0 - (cx[:, :, None] + sL)
    assert e_x.min() >= -1 and e_x.max() <= 0, (e_x.min(), e_x.max())
    pi = (xstart - 2 * pair)[:, :, None]
    t0 = pi + e_x + 1
    hh, ww, rr = np.meshgrid(np.arange(H), np.arange(W), np.arange(9),
                             indexing="ij")
    qq = rr if sign > 0 else 8 - rr
    wx4 = np.zeros((H, W, 9, 4), np.float32)
    wx4[hh, ww, qq, t0] = 1.0 - fx
    wx4[hh, ww, qq, t0 + 1] = fx

    e_y = y0 - (cy[:, :, None] + sL)
    assert e_y.min() >= -1 and e_y.max() <= 0, (e_y.min(), e_y.max())
    wy3 = np.zeros((H, W, 9, 3), np.float32)
    if sign > 0:
        wy3[hh, ww, rr, e_y + 1] = 1.0 - fy
        wy3[hh, ww, rr, e_y + 2] = fy
        idx0 = (cy - 5 + YPAD) * NPAIR + pair
        idxstep = NPAIR
    else:
        wy3[hh, ww, rr, 1 - e_y] = 1.0 - fy
        wy3[hh, ww, rr, -e_y] = fy
        idx0 = (cy + 5 + YPAD) * NPAIR + pair
        idxstep = -NPAIR
    rlo = idx0 + (10 * idxstep if idxstep < 0 else 0)
    rhi = idx0 + (10 * idxstep if idxstep > 0 else 0)
    assert rlo.min() >= 0 and rhi.max() < NROWS, (rlo.min(), rhi.max())
    assert xstart.min() >= 0 and (2 * pair + 12).max() <= XT

    inbx = ((x0 >= 0) & (x0 <= W - 1)).astype(np.float32)
    inbx1 = ((x0 + 1 >= 0) & (x0 + 1 <= W - 1)).astype(np.float32)
    inby = ((y0 >= 0) & (y0 <= H - 1)).astype(np.float32)
    inby1 = ((y0 + 1 >= 0) & (y0 + 1 <= H - 1)).astype(np.float32)
    mx = (1 - fx) * inbx + fx * inbx1
    my = (1 - fy) * inby + fy * inby1
    m2 = mx[:, :, None, :] * my[:, :, :, None]        # [H, W, dv, du]
    mbin = np.where(m2 < np.float32(0.999), np.float32(0), np.float32(1))
    return dict(wx4=wx4, wy3=wy3, idx0=idx0, idxstep=idxstep, mask=mbin)


def make_in_maps(feature1, feature2, BM):
    f1 = np.ascontiguousarray(np.asarray(feature1, dtype=np.float32))
    f2 = np.ascontiguousarray(np.asarray(feature2, dtype=np.float32))
    bm = np.asarray(BM, dtype=np.float32)

    fields = {}
    for b in range(B):
        fields[(b, +1)] = _host_fields(bm, +1, b)
        fields[(b, -1)] = _host_fields(bm, -1, b)

    in_maps = []
    groups_per_core = []
    for k in range(NCORES):
        gs = list(range(GPC * k, GPC * (k + 1)))
        groups_per_core.append(gs)
        b0 = gs[0] // H
        assert all(g // H == b0 for g in gs)
        wxy = np.zeros((GPC, 128, 126), np.float32)
        msk = np.zeros((GPC, 128, R), np.float32)
        gidx = np.zeros((GPC, 2, 128, 88), np.int16)
        for gi, g in enumerate(gs):
            h = g % H
            for wi, sign in enumerate((+1, -1)):
                fl = fields[(b0, sign)]
                wxy[gi, :, 63 * wi:63 * wi + 36] = \
                    fl["wx4"][h].reshape(128, 36)
                wxy[gi, :, 63 * wi + 36:63 * wi + 63] = \
                    fl["wy3"][h].reshape(128, 27)
                rows = (fl["idx0"][h][None, :]
                        + np.arange(NW)[:, None] * fl["idxstep"])  # [11, 128]
                wrapped = rows.reshape(-1).astype(np.int16)
                wrapped = wrapped.reshape(88, 16).T      # [16, 88]
                gidx[gi, wi] = np.tile(wrapped, (8, 1))
            m = (fields[(b0, +1)]["mask"][h]
                 * fields[(b0, -1)]["mask"][h])          # [W, dv, du]
            msk[gi] = m.reshape(128, R)
        in_maps.append({
            "f2b": f2[b0], "f1b": f1[b0],
            "wxy": wxy, "msk": msk, "gidx": gidx,
        })
    return in_maps, groups_per_core, None


_NC_CACHE = {}


def get_program():
    if "nc" not in _NC_CACHE:
        _NC_CACHE["nc"] = build_program()
    return _NC_CACHE["nc"]


def assemble_output(results, groups_per_core, _unused=None):
    out = np.zeros((B, R, H, W), np.float32)
    for k in range(NCORES):
        core_out = results[k]["out"]          # [GPC, 128, R]
        for gi, g in enumerate(groups_per_core[k]):
            b, h = g // H, g % H
            out[b, :, h, :] = core_out[gi].T
    return out


def kernel(feature1, feature2, BM):
    nc = get_program()
    in_maps, groups_per_core, _ = make_in_maps(feature1, feature2, BM)
    res = bass_utils.run_bass_kernel_spmd(
        nc, in_maps, core_ids=list(range(NCORES)))
    return assemble_output(res.results, groups_per_core)
